# revision 10
# baseline (speedup 1.0000x reference)
"""Trainium2 Bass kernel for nn_Conv1dMultiscaleLocalization.

Problem (per batch image [768,768], B=8, one image per NeuronCore):
  resp_j = vconv(C, k_j) + hconv(S, k_j)   j=0..6, 65-tap +-1/0 kernels
  conv_resp = max_j resp_j ; pos = relu(conv_resp)
  pooled = 11x11 stride-1 max pool (-inf pad)
  mask = (pos == pooled) & (pos > 0.5)
Returns (conv_resp [8,1,768,768] f32, mask [8,1,768,768] bool).

Device algorithm:
  - Both 1D convs as PE (tensor engine) matmuls with unscaled +-1 Toeplitz
    weights in bf16; data split exactly into bf16 hi+lo (2 passes) so
    products are exact and PSUM accumulates in fp32 (error ~2^-18).
  - V-dir: Toeplitz stationary [K=128,M=64], moving = C row-blocks
    (two host-prepadded row-offset copies, -32 and +32).
  - H-dir: stationary = S^T data chunks [128,128], moving = Toeplitz band
    template; output lands in the SAME natural-orientation PSUM tile, so
    resp_j = DV_j + DH_j accumulates for free in PSUM.
  - Combine: per-j scaled max chain on ACT (j=0 copy*s) + DVE
    scalar_tensor_tensor (mult, max) eviction.
  - 11x11 pool separable: horizontal max-window chain on DVE/Pool in
    natural layout; vertical via PE transpose -> free-dim chain -> PE
    transpose back.  All fp32-exact so pos==pooled semantics match numpy.
"""
import sys
import numpy as np

sys.path.insert(0, "/opt/trn_rl_repo")

import ml_dtypes  # noqa: E402
import concourse.bacc as bacc  # noqa: E402
import concourse.mybir as mybir  # noqa: E402
import concourse.tile as tile  # noqa: E402
from concourse.bass_utils import run_bass_kernel_spmd  # noqa: E402

F32 = mybir.dt.float32
BF16 = mybir.dt.bfloat16
U8 = mybir.dt.uint8
AF = mybir.ActivationFunctionType
ALU = mybir.AluOpType

H = W = 768
KERNEL_SIZES = [3, 9, 15, 21, 31, 51, 65]
NJ = 7
SCALES = [1.0 / (w - 1) for w in KERNEL_SIZES]
NB = 6          # 128-row blocks per image
NEG = -3.0e38   # -inf surrogate for max-pool padding
NTERMS = 2      # bf16 split terms (hi, lo)

_CACHE = {}


# ---------------------------------------------------------------- constants
def _sign_band(d, x):
    return np.where((d >= -x) & (d <= -1), 1.0,
                    np.where((d >= 1) & (d <= x), -1.0, 0.0))


def _toeplitz_va():
    """[128, 7*128]: out rows [128b,+128), K = c96 block b rows (abs -32+u).
    TVa[u, il] = sign(u - 32 - il)."""
    T = np.zeros((128, NJ * 128), dtype=np.float32)
    for j, w in enumerate(KERNEL_SIZES):
        x = (w - 1) // 2
        d = np.arange(128)[:, None] - 32 - np.arange(128)[None, :]
        T[:, 128 * j:128 * (j + 1)] = _sign_band(d, x)
    return T


def _toeplitz_vb():
    """[64, 7*128]: K = c96 block b+1 rows [0:64) = C rows 128b+96+u2.
    d = (128b+96+u2) - (128b+il) -> TVb[u2, il] = sign(u2 + 96 - il)."""
    T = np.zeros((64, NJ * 128), dtype=np.float32)
    for j, w in enumerate(KERNEL_SIZES):
        x = (w - 1) // 2
        d = np.arange(64)[:, None] + 96 - np.arange(128)[None, :]
        T[:, 128 * j:128 * (j + 1)] = _sign_band(d, x)
    return T


def _band_h():
    """[128, 7*192] moving H template: T[ul, 192j+wl] = sign(ul-(wl-32)) in band.

    For u-block b: u = 128b+ul contributes to w = 128b-32+wl with weight
    k_j(u-w) where k_j(d)=+1 for -x<=d<=-1, -1 for 1<=d<=x."""
    T = np.zeros((128, NJ * 192), dtype=np.float32)
    for j, w in enumerate(KERNEL_SIZES):
        x = (w - 1) // 2
        ul = np.arange(128)[:, None]
        wl = np.arange(192)[None, :]
        dd = ul - (wl - 32)
        T[:, 192 * j:192 * (j + 1)] = np.where(
            (dd >= -x) & (dd <= -1), 1.0, np.where((dd >= 1) & (dd <= x), -1.0, 0.0))
    return T


def _split_terms(x):
    """Exact-ish bf16 decomposition x ~= sum(terms); NTERMS bf16 arrays."""
    terms = []
    r = x
    for _ in range(NTERMS):
        t = r.astype(ml_dtypes.bfloat16)
        terms.append(t)
        r = r - t.astype(np.float32)
    return terms


# ---------------------------------------------------------------- kernel IR
def _build():
    nc = bacc.Bacc()
    ins = {}
    for t in range(NTERMS):
        ins[f"c96_{t}"] = nc.declare_dram_parameter(
            f"c96_{t}", [7 * 128, W], BF16, isOutput=False)
        ins[f"st_{t}"] = nc.declare_dram_parameter(
            f"st_{t}", [H, W], BF16, isOutput=False)
    TVA = nc.declare_dram_parameter("TVA", [128, NJ * 128], BF16, isOutput=False)
    TVB2 = nc.declare_dram_parameter("TVB2", [64, NJ * 128], BF16, isOutput=False)
    THB = nc.declare_dram_parameter("THB", [128, NJ * 192], BF16, isOutput=False)
    IDT = nc.declare_dram_parameter("IDT", [128, 128], F32, isOutput=False)
    CONV = nc.declare_dram_parameter("conv", [H, W], F32, isOutput=True)
    MASK = nc.declare_dram_parameter("mask", [H, W], U8, isOutput=True)

    with tile.TileContext(nc) as tc:
        with tc.tile_pool(name="big", bufs=1) as big, \
             tc.tile_pool(name="consts", bufs=1) as cst, \
             tc.tile_pool(name="conv", bufs=3) as convp, \
             tc.tile_pool(name="posg", bufs=1) as posp, \
             tc.tile_pool(name="pool", bufs=2) as poolp, \
             tc.tile_pool(name="atg", bufs=1) as atgp, \
             tc.tile_pool(name="pooled", bufs=1) as pooledp, \
             tc.tile_pool(name="small", bufs=2) as smallp, \
             tc.tile_pool(name="ps", bufs=8, space="PSUM") as ps:

            # ---- loads (tiles [128, 6*768]; dram row 128t+p -> col 768t+f)
            sb = {}
            for key, dram in ins.items():
                nt = dram.shape[0] // 128
                tl = big.tile([128, nt * W], BF16, tag=key, name=key)
                nc.sync.dma_start(
                    out=tl[:].rearrange("p (t f) -> p t f", t=nt),
                    in_=dram.rearrange("(t p) f -> p t f", p=128))
                sb[key] = tl
            tva = cst.tile([128, NJ * 128], BF16, tag="tva")
            tvb2 = cst.tile([64, NJ * 128], BF16, tag="tvb2")
            thb = cst.tile([128, NJ * 192], BF16, tag="thb")
            idt = cst.tile([128, 128], F32, tag="idt")
            nc.sync.dma_start(out=tva[:], in_=TVA[:])
            nc.sync.dma_start(out=tvb2[:], in_=TVB2[:])
            nc.sync.dma_start(out=thb[:], in_=THB[:])
            nc.sync.dma_start(out=idt[:], in_=IDT[:])

            # ---- conv waves: (ib, half, jgroup)
            posg = [posp.tile([128, 800], F32, tag=f"posg{ib}", name=f"posg{ib}") for ib in range(NB)]
            JG = [(0, 4), (4, 7)]
            for ib in range(NB):
                cv = convp.tile([128, W], F32, tag="cv")
                for h in range(2):
                    wlo_h, whi_h = 384 * h, 384 * (h + 1)
                    segs = []
                    for ub in range(NB):
                        lo = max(0, 128 * ub - 32, wlo_h)
                        hi = min(W, 128 * ub + 160, whi_h)
                        if lo < hi:
                            segs.append((ub, lo, hi))
                    for (j0, j1) in JG:
                        ptiles = {j: ps.tile([128, 384], F32, tag="p", name=f"p{j}")
                                  for j in range(j0, j1)}
                        # V first: K-split, M=128 full-region writes.
                        # A: K=128 from c96 block ib; B: K=64 from c96 block
                        # ib+1 partitions [0:64).  start=True on the first
                        # (opens group, lazy-zeroes the bank region).
                        for j in range(j0, j1):
                            p = ptiles[j]
                            for t in range(NTERMS):
                                rhs = sb[f"c96_{t}"][:, W * ib + wlo_h:W * ib + whi_h]
                                nc.tensor.matmul(
                                    p[:], tva[:, 128 * j:128 * (j + 1)], rhs,
                                    start=(t == 0), stop=False)
                            for t in range(NTERMS):
                                rhs = sb[f"c96_{t}"][0:64, W * (ib + 1) + wlo_h:
                                                     W * (ib + 1) + whi_h]
                                nc.tensor.matmul(
                                    p[:], tvb2[0:64, 128 * j:128 * (j + 1)], rhs,
                                    start=False, stop=False)
                        # H: accumulate u-block contributions (overlaps fine,
                        # every byte already written by V)
                        for si, (ub, lo, hi) in enumerate(segs):
                            for t in range(NTERMS):
                                lhs_d = sb[f"st_{t}"][:, W * ub + 128 * ib:
                                                      W * ub + 128 * (ib + 1)]
                                last = (si == len(segs) - 1) and (t == NTERMS - 1)
                                for j in range(j0, j1):
                                    off = 192 * j + (lo - (128 * ub - 32))
                                    rhs = thb[:, off:off + (hi - lo)]
                                    nc.tensor.matmul(
                                        ptiles[j][:, lo - wlo_h:hi - wlo_h],
                                        lhs_d, rhs,
                                        start=False, stop=last)
                        # combine into conv tile
                        for j in range(j0, j1):
                            p = ptiles[j]
                            dst = cv[:, wlo_h:whi_h]
                            if j == 0:
                                nc.scalar.activation(dst, p[:], AF.Copy,
                                                     scale=float(SCALES[0]))
                            else:
                                nc.vector.scalar_tensor_tensor(
                                    dst, p[:], float(SCALES[j]), dst,
                                    ALU.mult, ALU.max)
                nc.sync.dma_start(out=CONV[128 * ib:128 * (ib + 1), :], in_=cv[:])
                nc.scalar.activation(posg[ib][:, 16:784], cv[:], AF.Relu)
                nc.vector.memset(posg[ib][:, 0:16], NEG)
                nc.vector.memset(posg[ib][:, 784:800], NEG)

            # ---- pool horizontal (natural; window 11 centered)
            # A[w] = max(pos[w-5..w+5]) = M11[w+11] with cols offset 16
            atg = [atgp.tile([128, 800], F32, tag=f"atg{c}", name=f"atg{c}") for c in range(NB)]
            for c in range(NB):
                nc.vector.memset(atg[c][:, 0:16], NEG)
                nc.vector.memset(atg[c][:, 784:800], NEG)
            for ib in range(NB):
                m2 = poolp.tile([128, 800], F32, tag="m2")
                m4 = poolp.tile([128, 800], F32, tag="m4")
                m8 = poolp.tile([128, 800], F32, tag="m8")
                a = poolp.tile([128, W], F32, tag="a")
                g = posg[ib]
                nc.vector.tensor_tensor(m2[:, 0:799], g[:, 0:799], g[:, 1:800], ALU.max)
                nc.vector.tensor_tensor(m4[:, 0:797], m2[:, 0:797], m2[:, 2:799], ALU.max)
                nc.vector.tensor_tensor(m8[:, 0:793], m4[:, 0:793], m4[:, 4:797], ALU.max)
                nc.vector.tensor_tensor(a[:], m8[:, 11:779], m4[:, 18:786], ALU.max)
                # transpose A -> atg columns (i block ib), 3 chunks per psum tile
                for half in range(2):
                    pt = ps.tile([128, 384], F32, tag="p", name="pt")
                    for k in range(3):
                        c = 3 * half + k
                        nc.tensor.transpose(pt[:, 128 * k:128 * (k + 1)],
                                            a[:, 128 * c:128 * (c + 1)], idt[:])
                    for k in range(3):
                        c = 3 * half + k
                        nc.scalar.activation(
                            atg[c][:, 16 + 128 * ib:16 + 128 * (ib + 1)],
                            pt[:, 128 * k:128 * (k + 1)], AF.Copy)

            # ---- pool vertical (transposed space) + transpose back + mask
            pooled = [None] * NB
            for ib in range(NB):
                pooled[ib] = pooledp.tile([128, W], F32, tag=f"pl{ib}", name=f"pl{ib}")
            ptt = [None] * NB
            for c in range(NB):
                m2 = poolp.tile([128, 800], F32, tag="m2")
                m4 = poolp.tile([128, 800], F32, tag="m4")
                m8 = poolp.tile([128, 800], F32, tag="m8")
                ptv = poolp.tile([128, W], F32, tag="ptv")
                g = atg[c]
                nc.vector.tensor_tensor(m2[:, 0:799], g[:, 0:799], g[:, 1:800], ALU.max)
                nc.vector.tensor_tensor(m4[:, 0:797], m2[:, 0:797], m2[:, 2:799], ALU.max)
                nc.vector.tensor_tensor(m8[:, 0:793], m4[:, 0:793], m4[:, 4:797], ALU.max)
                nc.vector.tensor_tensor(ptv[:], m8[:, 11:779], m4[:, 18:786], ALU.max)
                # transpose back: ptv [w-part c, i] -> pooled[ib][:, 128c..]
                for half in range(2):
                    pt = ps.tile([128, 384], F32, tag="p", name="pt")
                    for k in range(3):
                        ib = 3 * half + k
                        nc.tensor.transpose(pt[:, 128 * k:128 * (k + 1)],
                                            ptv[:, 128 * ib:128 * (ib + 1)], idt[:])
                    for k in range(3):
                        ib = 3 * half + k
                        nc.scalar.activation(
                            pooled[ib][:, 128 * c:128 * (c + 1)],
                            pt[:, 128 * k:128 * (k + 1)], AF.Copy)
            for ib in range(NB):
                eq = smallp.tile([128, W], F32, tag="eq")
                mk = smallp.tile([128, W], U8, tag="mk")
                nc.vector.tensor_tensor(eq[:], posg[ib][:, 16:784], pooled[ib][:],
                                        ALU.is_equal)
                nc.vector.scalar_tensor_tensor(
                    mk[:], posg[ib][:, 16:784], 0.5, eq[:],
                    ALU.is_gt, ALU.logical_and)
                nc.sync.dma_start(out=MASK[128 * ib:128 * (ib + 1), :], in_=mk[:])

    nc.compile()
    return nc


# ---------------------------------------------------------------- host glue
def _prep_core(Cb, Sb):
    """Per-image host prep: padded row-offset copies + bf16 splits."""
    c96 = np.vstack([np.zeros((32, W), np.float32), Cb,
                     np.zeros((96, W), np.float32)])   # 7 blocks, rows -32..
    st = np.ascontiguousarray(Sb.T)
    m = {}
    for nm, arr in (("c96", c96), ("st", st)):
        for t, term in enumerate(_split_terms(arr)):
            m[f"{nm}_{t}"] = term
    return m


def kernel(C, S, kernel_cos, kernel_sin):
    C = np.asarray(C, dtype=np.float32)
    S = np.asarray(S, dtype=np.float32)
    B = C.shape[0]
    if "nc" not in _CACHE:
        _CACHE["nc"] = _build()
    nc = _CACHE["nc"]
    consts = {
        "TVA": _toeplitz_va().astype(ml_dtypes.bfloat16),
        "TVB2": _toeplitz_vb().astype(ml_dtypes.bfloat16),
        "THB": _band_h().astype(ml_dtypes.bfloat16),
        "IDT": np.eye(128, dtype=np.float32),
    }
    in_maps = []
    for b in range(B):
        m = _prep_core(C[b, 0], S[b, 0])
        m.update(consts)
        in_maps.append(m)
    res = run_bass_kernel_spmd(nc, in_maps, core_ids=list(range(B)))
    conv = np.stack([r["conv"] for r in res.results])[:, None]
    mask = np.stack([r["mask"] for r in res.results])[:, None].astype(bool)
    return conv.astype(np.float32), mask


# revision 13
# speedup vs baseline: 1.0869x; 1.0869x over previous
"""Trainium2 Bass kernel for nn_Conv1dMultiscaleLocalization.

Problem (per batch image [768,768], B=8, one image per NeuronCore):
  resp_j = vconv(C, k_j) + hconv(S, k_j)   j=0..6, 65-tap +-1/0 kernels
  conv_resp = max_j resp_j ; pos = relu(conv_resp)
  pooled = 11x11 stride-1 max pool (-inf pad)
  mask = (pos == pooled) & (pos > 0.5)
Returns (conv_resp [8,1,768,768] f32, mask [8,1,768,768] bool).

Device algorithm:
  - Both 1D convs as PE (tensor engine) matmuls with unscaled +-1 Toeplitz
    weights in bf16; data split exactly into bf16 hi+lo (2 passes) so
    products are exact and PSUM accumulates in fp32 (error ~2^-18).
  - V-dir: Toeplitz stationary [K=128,M=64], moving = C row-blocks
    (two host-prepadded row-offset copies, -32 and +32).
  - H-dir: stationary = S^T data chunks [128,128], moving = Toeplitz band
    template; output lands in the SAME natural-orientation PSUM tile, so
    resp_j = DV_j + DH_j accumulates for free in PSUM.
  - Combine: per-j scaled max chain on ACT (j=0 copy*s) + DVE
    scalar_tensor_tensor (mult, max) eviction.
  - 11x11 pool separable: horizontal max-window chain on DVE/Pool in
    natural layout; vertical via PE transpose -> free-dim chain -> PE
    transpose back.  All fp32-exact so pos==pooled semantics match numpy.
"""
import sys
import numpy as np

sys.path.insert(0, "/opt/trn_rl_repo")

import ml_dtypes  # noqa: E402
import concourse.bacc as bacc  # noqa: E402
import concourse.mybir as mybir  # noqa: E402
import concourse.tile as tile  # noqa: E402
from concourse.bass_utils import run_bass_kernel_spmd  # noqa: E402

F32 = mybir.dt.float32
BF16 = mybir.dt.bfloat16
U8 = mybir.dt.uint8
AF = mybir.ActivationFunctionType
ALU = mybir.AluOpType

H = W = 768
KERNEL_SIZES = [3, 9, 15, 21, 31, 51, 65]
NJ = 7
SCALES = [1.0 / (w - 1) for w in KERNEL_SIZES]
NB = 6          # 128-row blocks per image
NEG = -3.0e38   # -inf surrogate for max-pool padding
NTERMS = 2      # bf16 split terms (hi, lo)

_CACHE = {}


# ---------------------------------------------------------------- constants
def _sign_band(d, x):
    return np.where((d >= -x) & (d <= -1), 1.0,
                    np.where((d >= 1) & (d <= x), -1.0, 0.0))


def _toeplitz_va():
    """[128, 7*128]: out rows [128b,+128), K = c96 block b rows (abs -32+u).
    TVa[u, il] = sign(u - 32 - il)."""
    T = np.zeros((128, NJ * 128), dtype=np.float32)
    for j, w in enumerate(KERNEL_SIZES):
        x = (w - 1) // 2
        d = np.arange(128)[:, None] - 32 - np.arange(128)[None, :]
        T[:, 128 * j:128 * (j + 1)] = _sign_band(d, x)
    return T


def _toeplitz_vb():
    """[64, 7*128]: K = c96 block b+1 rows [0:64) = C rows 128b+96+u2.
    d = (128b+96+u2) - (128b+il) -> TVb[u2, il] = sign(u2 + 96 - il)."""
    T = np.zeros((64, NJ * 128), dtype=np.float32)
    for j, w in enumerate(KERNEL_SIZES):
        x = (w - 1) // 2
        d = np.arange(64)[:, None] + 96 - np.arange(128)[None, :]
        T[:, 128 * j:128 * (j + 1)] = _sign_band(d, x)
    return T


def _band_h():
    """[128, 7*192] moving H template: T[ul, 192j+wl] = sign(ul-(wl-32)) in band.

    For u-block b: u = 128b+ul contributes to w = 128b-32+wl with weight
    k_j(u-w) where k_j(d)=+1 for -x<=d<=-1, -1 for 1<=d<=x."""
    T = np.zeros((128, NJ * 192), dtype=np.float32)
    for j, w in enumerate(KERNEL_SIZES):
        x = (w - 1) // 2
        ul = np.arange(128)[:, None]
        wl = np.arange(192)[None, :]
        dd = ul - (wl - 32)
        T[:, 192 * j:192 * (j + 1)] = np.where(
            (dd >= -x) & (dd <= -1), 1.0, np.where((dd >= 1) & (dd <= x), -1.0, 0.0))
    return T


def _split_terms(x):
    """Exact-ish bf16 decomposition x ~= sum(terms); NTERMS bf16 arrays."""
    terms = []
    r = x
    for _ in range(NTERMS):
        t = r.astype(ml_dtypes.bfloat16)
        terms.append(t)
        r = r - t.astype(np.float32)
    return terms


# ---------------------------------------------------------------- kernel IR
def _build():
    nc = bacc.Bacc()
    ins = {}
    for t in range(NTERMS):
        ins[f"c96_{t}"] = nc.declare_dram_parameter(
            f"c96_{t}", [7 * 128, W], BF16, isOutput=False)
        ins[f"st_{t}"] = nc.declare_dram_parameter(
            f"st_{t}", [H, W], BF16, isOutput=False)
    TVA = nc.declare_dram_parameter("TVA", [128, NJ * 128], BF16, isOutput=False)
    TVB2 = nc.declare_dram_parameter("TVB2", [64, NJ * 128], BF16, isOutput=False)
    THB = nc.declare_dram_parameter("THB", [128, NJ * 192], BF16, isOutput=False)
    IDT = nc.declare_dram_parameter("IDT", [128, 128], F32, isOutput=False)
    CONV = nc.declare_dram_parameter("conv", [H, W], F32, isOutput=True)
    MASK = nc.declare_dram_parameter("mask", [H, W], U8, isOutput=True)

    with tile.TileContext(nc) as tc:
        with tc.tile_pool(name="big", bufs=1) as big, \
             tc.tile_pool(name="consts", bufs=1) as cst, \
             tc.tile_pool(name="conv", bufs=3) as convp, \
             tc.tile_pool(name="posg", bufs=1) as posp, \
             tc.tile_pool(name="pool", bufs=2) as poolp, \
             tc.tile_pool(name="atg", bufs=1) as atgp, \
             tc.tile_pool(name="pooled", bufs=1) as pooledp, \
             tc.tile_pool(name="small", bufs=2) as smallp, \
             tc.tile_pool(name="ps", bufs=8, space="PSUM") as ps:

            # ---- loads (tiles [128, 6*768]; dram row 128t+p -> col 768t+f)
            sb = {}
            for key, dram in ins.items():
                nt = dram.shape[0] // 128
                tl = big.tile([128, nt * W], BF16, tag=key, name=key)
                # per-block DMAs spread across queues and unblock early waves
                for t in range(nt):
                    nc.sync.dma_start(out=tl[:, W * t:W * (t + 1)],
                                      in_=dram[128 * t:128 * (t + 1), :])
                sb[key] = tl
            tva = cst.tile([128, NJ * 128], BF16, tag="tva")
            tvb2 = cst.tile([64, NJ * 128], BF16, tag="tvb2")
            thb = cst.tile([128, NJ * 192], BF16, tag="thb")
            idt = cst.tile([128, 128], F32, tag="idt")
            nc.sync.dma_start(out=tva[:], in_=TVA[:])
            nc.sync.dma_start(out=tvb2[:], in_=TVB2[:])
            nc.sync.dma_start(out=thb[:], in_=THB[:])
            nc.sync.dma_start(out=idt[:], in_=IDT[:])

            # ---- fused conv + pipelined pool ----------------------------
            # posg[ib]: guarded raw conv tile [128, 800], data at [16:784).
            # relu is folded away: mw(relu(x)) = relu(mw(x)) and the mask
            # (pos==pooled)&(pos>0.5) equals (conv==mw(conv))&(conv>0.5).
            posg = [posp.tile([128, 800], F32, tag=f"posg{ib}", name=f"posg{ib}")
                    for ib in range(NB)]
            atg = [atgp.tile([128, 800], F32, tag=f"atg{c}", name=f"atg{c}")
                   for c in range(NB)]
            ptv = [pooledp.tile([128, W], F32, tag=f"ptv{c}", name=f"ptv{c}")
                   for c in range(NB)]
            for c in range(NB):
                nc.vector.memset(atg[c][:, 0:16], NEG)
                nc.vector.memset(atg[c][:, 784:800], NEG)
            XJ = [(w - 1) // 2 for w in KERNEL_SIZES]
            JG = [(0, 4), (4, 7)]

            def emit_wave(ib, h):
                wlo_h, whi_h = 384 * h, 384 * (h + 1)
                for (j0, j1) in JG:
                    ptiles = {j: ps.tile([128, 384], F32, tag="p", name=f"p{j}")
                              for j in range(j0, j1)}
                    # V first: K-split, M=128 full-region writes; the first
                    # opens the group (lazy-zeroes the bank region).
                    for j in range(j0, j1):
                        p = ptiles[j]
                        for t in range(NTERMS):
                            rhs = sb[f"c96_{t}"][:, W * ib + wlo_h:W * ib + whi_h]
                            nc.tensor.matmul(
                                p[:], tva[:, 128 * j:128 * (j + 1)], rhs,
                                start=(t == 0), stop=False)
                        for t in range(NTERMS):
                            rhs = sb[f"c96_{t}"][0:64, W * (ib + 1) + wlo_h:
                                                 W * (ib + 1) + whi_h]
                            nc.tensor.matmul(
                                p[:], tvb2[0:64, 128 * j:128 * (j + 1)], rhs,
                                start=False, stop=False)
                    # H accumulates; per-j halo = x_j, so narrow kernels
                    # stream fewer columns.  Stationary S^T chunk shared by j.
                    spans = {}
                    for ub in range(NB):
                        for j in range(j0, j1):
                            lo = max(0, 128 * ub - XJ[j], wlo_h)
                            hi = min(W, 128 * ub + 128 + XJ[j], whi_h)
                            if lo < hi:
                                spans[(ub, j)] = (lo, hi)
                    lasts = {}
                    for (ub, j) in spans:
                        lasts[j] = ub
                    for ub in range(NB):
                        if not any((ub, j) in spans for j in range(j0, j1)):
                            continue
                        for t in range(NTERMS):
                            lhs_d = sb[f"st_{t}"][:, W * ub + 128 * ib:
                                                  W * ub + 128 * (ib + 1)]
                            for j in range(j0, j1):
                                if (ub, j) not in spans:
                                    continue
                                lo, hi = spans[(ub, j)]
                                off = 192 * j + (lo - (128 * ub - 32))
                                rhs = thb[:, off:off + (hi - lo)]
                                nc.tensor.matmul(
                                    ptiles[j][:, lo - wlo_h:hi - wlo_h],
                                    lhs_d, rhs, start=False,
                                    stop=(lasts[j] == ub and t == NTERMS - 1))
                    # combine into guarded conv tile
                    for j in range(j0, j1):
                        dst = posg[ib][:, 16 + wlo_h:16 + whi_h]
                        if j == 0:
                            nc.scalar.activation(dst, ptiles[j][:], AF.Copy,
                                                 scale=float(SCALES[0]))
                        else:
                            nc.vector.scalar_tensor_tensor(
                                dst, ptiles[j][:], float(SCALES[j]), dst,
                                ALU.mult, ALU.max)

            def emit_mwh(ib):
                # A[w] = max(conv[w-5..w+5]); A cols natural [0,768)
                nc.vector.memset(posg[ib][:, 0:16], NEG)
                nc.vector.memset(posg[ib][:, 784:800], NEG)
                m2 = poolp.tile([128, 800], F32, tag="m2", name="m2")
                m4 = poolp.tile([128, 800], F32, tag="m4", name="m4")
                m8 = poolp.tile([128, 800], F32, tag="m8", name="m8")
                a = poolp.tile([128, W], F32, tag="a", name="a")
                g = posg[ib]
                nc.vector.tensor_tensor(m2[:, 0:799], g[:, 0:799], g[:, 1:800], ALU.max)
                nc.vector.tensor_tensor(m4[:, 0:797], m2[:, 0:797], m2[:, 2:799], ALU.max)
                nc.vector.tensor_tensor(m8[:, 0:793], m4[:, 0:793], m4[:, 4:797], ALU.max)
                nc.vector.tensor_tensor(a[:], m8[:, 11:779], m4[:, 18:786], ALU.max)
                return a

            def emit_at(ib, a):
                # transpose A(ib) -> atg[c] column block ib
                for half in range(2):
                    pt = ps.tile([128, 384], F32, tag="p", name="pt")
                    for k in range(3):
                        c = 3 * half + k
                        nc.tensor.transpose(pt[:, 128 * k:128 * (k + 1)],
                                            a[:, 128 * c:128 * (c + 1)], idt[:])
                    for k in range(3):
                        c = 3 * half + k
                        nc.scalar.activation(
                            atg[c][:, 16 + 128 * ib:16 + 128 * (ib + 1)],
                            pt[:, 128 * k:128 * (k + 1)], AF.Copy)

            def emit_poolv(vib):
                # vertical max window on atg columns [128vib, +128) (+-8 halo)
                # local coords: l = col - (av-8), scratch [128, 144]
                av = 16 + 128 * vib
                for c in range(NB):
                    m2 = poolp.tile([128, 144], F32, tag="m2v", name="m2v")
                    m4 = poolp.tile([128, 144], F32, tag="m4v", name="m4v")
                    m8 = poolp.tile([128, 144], F32, tag="m8v", name="m8v")
                    g = atg[c]
                    nc.vector.tensor_tensor(m2[:, 0:144],
                                            g[:, av - 8:av + 136],
                                            g[:, av - 7:av + 137], ALU.max)
                    nc.vector.tensor_tensor(m4[:, 0:142],
                                            m2[:, 0:142], m2[:, 2:144], ALU.max)
                    nc.vector.tensor_tensor(m8[:, 2:138],
                                            m4[:, 2:138], m4[:, 6:142], ALU.max)
                    nc.vector.tensor_tensor(ptv[c][:, 128 * vib:128 * (vib + 1)],
                                            m8[:, 3:131], m4[:, 10:138], ALU.max)
                plv = smallp.tile([128, W], F32, tag="plv", name="plv")
                for half in range(2):
                    pt = ps.tile([128, 384], F32, tag="p", name="pt")
                    for k in range(3):
                        c = 3 * half + k
                        nc.tensor.transpose(pt[:, 128 * k:128 * (k + 1)],
                                            ptv[c][:, 128 * vib:128 * (vib + 1)],
                                            idt[:])
                    nc.scalar.activation(plv[:, 384 * half:384 * (half + 1)],
                                         pt[:], AF.Copy)
                eq = smallp.tile([128, W], F32, tag="eq", name="eq")
                mk = smallp.tile([128, W], U8, tag="mk", name="mk")
                nc.vector.tensor_tensor(eq[:], posg[vib][:, 16:784], plv[:],
                                        ALU.is_equal)
                nc.vector.scalar_tensor_tensor(
                    mk[:], posg[vib][:, 16:784], 0.5, eq[:],
                    ALU.is_gt, ALU.logical_and)
                nc.sync.dma_start(out=MASK[128 * vib:128 * (vib + 1), :], in_=mk[:])

            alist = {}
            for ib in range(NB):
                emit_wave(ib, 0)
                if ib >= 1:
                    emit_at(ib - 1, alist[ib - 1])
                emit_wave(ib, 1)
                nc.sync.dma_start(out=CONV[128 * ib:128 * (ib + 1), :],
                                  in_=posg[ib][:, 16:784])
                alist[ib] = emit_mwh(ib)
                if ib >= 2:
                    emit_poolv(ib - 2)
            emit_at(NB - 1, alist[NB - 1])
            emit_poolv(NB - 2)
            emit_poolv(NB - 1)

    nc.compile()
    return nc


# ---------------------------------------------------------------- host glue
def _prep_core(Cb, Sb):
    """Per-image host prep: padded row-offset copies + bf16 splits."""
    c96 = np.vstack([np.zeros((32, W), np.float32), Cb,
                     np.zeros((96, W), np.float32)])   # 7 blocks, rows -32..
    st = np.ascontiguousarray(Sb.T)
    m = {}
    for nm, arr in (("c96", c96), ("st", st)):
        for t, term in enumerate(_split_terms(arr)):
            m[f"{nm}_{t}"] = term
    return m


def kernel(C, S, kernel_cos, kernel_sin):
    C = np.asarray(C, dtype=np.float32)
    S = np.asarray(S, dtype=np.float32)
    B = C.shape[0]
    if "nc" not in _CACHE:
        _CACHE["nc"] = _build()
    nc = _CACHE["nc"]
    consts = {
        "TVA": _toeplitz_va().astype(ml_dtypes.bfloat16),
        "TVB2": _toeplitz_vb().astype(ml_dtypes.bfloat16),
        "THB": _band_h().astype(ml_dtypes.bfloat16),
        "IDT": np.eye(128, dtype=np.float32),
    }
    in_maps = []
    for b in range(B):
        m = _prep_core(C[b, 0], S[b, 0])
        m.update(consts)
        in_maps.append(m)
    res = run_bass_kernel_spmd(nc, in_maps, core_ids=list(range(B)))
    conv = np.stack([r["conv"] for r in res.results])[:, None]
    mask = np.stack([r["mask"] for r in res.results])[:, None].astype(bool)
    return conv.astype(np.float32), mask
